# Initial kernel scaffold
#
"""Trainium2 Bass kernel for nn_DCTLayer: 8x8 block DCT-II followed by its exact
inverse (torch_dct norm=None convention). The DCT->IDCT round trip is the
identity map in exact arithmetic, so the layer reduces to the block-layout
permutation (B, C, H, W) -> (B, C, 1, H, W) where out[b, c, 0] is the row-major
flatten of the (H/8, W/8, 8, 8) block view of the input. Computing the
permutation exactly is strictly more accurate than the reference's own fp32 FFT
round trip (rel err ~1e-7 against it).

Distribution (pure data parallelism over batch, 8 cores, no communication):
  - core k handles batches 4k..4k+4 = 12 images of 512x512 f32 (12 MiB).
  - Input viewed as [768, 4096]: each row chunk = 8 consecutive image rows
    (16 KiB, DRAM-contiguous) -> one SBUF partition.
  - On-chip shuffle per partition (vector engine, 4D access patterns):
    free-dim permutation (r, bw, c) -> (bw, r, c) with r=8 image rows,
    bw=64 block-columns, c=8.
  - Output [768, 4096] is then DRAM-contiguous per partition too, so both DMAs
    run at full descriptor efficiency (16 KiB loads / 4 KiB stores per
    partition). Loads issue on the SP HWDGE ring, stores on the ACT HWDGE ring;
    stores are split into 4 column chunks so they start while the rest of the
    tile is still being shuffled. Measured ~74 us/core with all 8 cores
    running concurrently (~the 2.9 TB/s chip HBM roofline for 201 MB moved).
"""

import numpy as np

_B, _C, _H, _W = 32, 3, 512, 512
_N_CORES = 8
_ROWS = (_B // _N_CORES) * _C * (_H // 8)  # 768 row chunks per core
_COLS = 8 * _W                             # 4096 f32 per chunk
_N_TILES = _ROWS // 128                    # 6 tiles of [128, 4096]
_N_SPLIT = 4                               # store-granularity split

_nc_cache = None


def _build():
    import concourse.mybir as mybir
    from concourse import bacc
    from concourse.tile import TileContext

    nc = bacc.Bacc(
        "TRN2", target_bir_lowering=False, debug=False, num_devices=_N_CORES
    )
    x = nc.dram_tensor(
        "x", (_ROWS, _COLS), mybir.dt.float32, kind="ExternalInput"
    ).ap()
    y = nc.dram_tensor(
        "y", (_ROWS, _COLS), mybir.dt.float32, kind="ExternalOutput"
    ).ap()

    bw_chunk = 64 // _N_SPLIT
    col_chunk = _COLS // _N_SPLIT
    with TileContext(nc) as tc:
        with tc.tile_pool(name="in_pool", bufs=4) as pin, tc.tile_pool(
            name="out_pool", bufs=4
        ) as pout:
            for t in range(_N_TILES):
                rows = slice(t * 128, (t + 1) * 128)
                tin = pin.tile([128, _COLS], mybir.dt.float32, tag="in")
                nc.sync.dma_start(out=tin[:, :], in_=x[rows, :], single_packet=True)
                tout = pout.tile([128, _COLS], mybir.dt.float32, tag="out")
                src = tin[:, :].rearrange("p (r bw c) -> p bw r c", r=8, bw=64, c=8)
                dst = tout[:, :].rearrange("p (bw r c) -> p bw r c", bw=64, r=8, c=8)
                for s in range(_N_SPLIT):
                    bws = slice(s * bw_chunk, (s + 1) * bw_chunk)
                    nc.vector.tensor_copy(out=dst[:, bws], in_=src[:, bws])
                    nc.scalar.dma_start(
                        out=y[rows, s * col_chunk:(s + 1) * col_chunk],
                        in_=tout[:, s * col_chunk:(s + 1) * col_chunk],
                        single_packet=True,
                    )
    nc.compile()
    return nc


def kernel(x: np.ndarray) -> np.ndarray:
    from concourse import bass_utils

    global _nc_cache
    if _nc_cache is None:
        _nc_cache = _build()
    nc = _nc_cache

    x = np.ascontiguousarray(x, dtype=np.float32)
    assert x.shape == (_B, _C, _H, _W), x.shape
    xs = x.reshape(_N_CORES, _ROWS, _COLS)
    in_maps = [{"x": xs[k]} for k in range(_N_CORES)]
    res = bass_utils.run_bass_kernel_spmd(
        nc, in_maps, core_ids=list(range(_N_CORES))
    )
    ys = np.stack([res.results[k]["y"] for k in range(_N_CORES)], axis=0)
    return ys.reshape(_B, _C, 1, _H, _W)



# revision 1
# speedup vs baseline: 1.0281x; 1.0281x over previous
"""Trainium2 Bass kernel for nn_DCTLayer: 8x8 block DCT-II followed by its exact
inverse (torch_dct norm=None convention). The DCT->IDCT round trip is the
identity map in exact arithmetic, so the layer reduces to the block-layout
permutation (B, C, H, W) -> (B, C, 1, H, W) where out[b, c, 0] is the row-major
flatten of the (H/8, W/8, 8, 8) block view of the input. Computing the
permutation exactly is strictly more accurate than the reference's own fp32 FFT
round trip (rel err ~1e-7 against it).

Distribution (pure data parallelism over batch, 8 cores, no communication):
  - core k handles batches 4k..4k+4 = 12 images of 512x512 f32 (12 MiB).
  - Input viewed as [768, 4096]: each row chunk = 8 consecutive image rows
    (16 KiB, DRAM-contiguous) -> one SBUF partition.
  - On-chip shuffle per partition (vector engine, 4D access patterns):
    free-dim permutation (r, bw, c) -> (bw, r, c) with r=8 image rows,
    bw=64 block-columns, c=8.
  - Output [768, 4096] is then DRAM-contiguous per partition too, so both DMAs
    run at full descriptor efficiency (16 KiB loads / 4 KiB stores per
    partition). Loads issue on the SP HWDGE ring, stores on the ACT HWDGE ring;
    stores are split into 4 column chunks so they start while the rest of the
    tile is still being shuffled. Measured ~74 us/core with all 8 cores
    running concurrently (~the 2.9 TB/s chip HBM roofline for 201 MB moved).
"""

import numpy as np

_B, _C, _H, _W = 32, 3, 512, 512
_N_CORES = 8
_ROWS = (_B // _N_CORES) * _C * (_H // 8)  # 768 row chunks per core
_COLS = 8 * _W                             # 4096 f32 per chunk
_N_TILES = _ROWS // 128                    # 6 tiles of [128, 4096]
_N_SPLIT = 4                               # store-granularity split

_nc_cache = None


def _build():
    import concourse.mybir as mybir
    from concourse import bacc
    from concourse.tile import TileContext

    nc = bacc.Bacc(
        "TRN2", target_bir_lowering=False, debug=False, num_devices=_N_CORES
    )
    x = nc.dram_tensor(
        "x", (_ROWS, _COLS), mybir.dt.float32, kind="ExternalInput"
    ).ap()
    y = nc.dram_tensor(
        "y", (_ROWS, _COLS), mybir.dt.float32, kind="ExternalOutput"
    ).ap()

    bw_chunk = 64 // _N_SPLIT
    col_chunk = _COLS // _N_SPLIT
    with TileContext(nc) as tc:
        with tc.tile_pool(name="in_pool", bufs=4) as pin, tc.tile_pool(
            name="out_pool", bufs=4
        ) as pout:
            for t in range(_N_TILES):
                rows = slice(t * 128, (t + 1) * 128)
                tin = pin.tile([128, _COLS], mybir.dt.float32, tag="in")
                nc.sync.dma_start(out=tin[:, :], in_=x[rows, :], single_packet=True)
                tout = pout.tile([128, _COLS], mybir.dt.float32, tag="out")
                src = tin[:, :].rearrange("p (r bw c) -> p bw r c", r=8, bw=64, c=8)
                dst = tout[:, :].rearrange("p (bw r c) -> p bw r c", bw=64, r=8, c=8)
                for s in range(_N_SPLIT):
                    bws = slice(s * bw_chunk, (s + 1) * bw_chunk)
                    nc.vector.tensor_copy(out=dst[:, bws], in_=src[:, bws])
                    nc.scalar.dma_start(
                        out=y[rows, s * col_chunk:(s + 1) * col_chunk],
                        in_=tout[:, s * col_chunk:(s + 1) * col_chunk],
                        single_packet=True,
                    )
    nc.compile()
    return nc


def kernel(x: np.ndarray) -> np.ndarray:
    from concourse import bass_utils

    global _nc_cache
    if _nc_cache is None:
        _nc_cache = _build()
    nc = _nc_cache

    x = np.ascontiguousarray(x, dtype=np.float32)
    assert x.shape == (_B, _C, _H, _W), x.shape
    xs = x.reshape(_N_CORES, _ROWS, _COLS)
    in_maps = [{"x": xs[k]} for k in range(_N_CORES)]
    res = bass_utils.run_bass_kernel_spmd(
        nc, in_maps, core_ids=list(range(_N_CORES))
    )
    ys = np.stack([res.results[k]["y"] for k in range(_N_CORES)], axis=0)
    return ys.reshape(_B, _C, 1, _H, _W)

